# revision 17
# baseline (speedup 1.0000x reference)
"""Trainium2 Bass kernel for nn_End2EndRVFixedOutput (nms_detection).

Reference semantics: out[100,7] starts at zeros; for n = 0..7 in order,
with off_n = (0 if n==0 else num_dets[n-1]) and k_n = num_dets[n],
rows [off_n, off_n+k_n) are overwritten with
[n, boxes[n,j,0:4], classes[n,j], scores[n,j]] for j = row-off_n.

num_dets < 12, so only the [:, :12] input slices matter and only out rows
0..21 can ever be written.  v3 device algorithm (per core, replicated):

  num_dets is DMAd into partitions 1..8 of a zeroed [9,1] column, so
  k_n and off_n = num_dets[n-1] are the same tile read at partition
  offsets 1 and 0 -- no shuffle, no casts (chain stays int32).

  d8p1[n,r] = r+1-off_n; rm8[n,r] = (0 < d8p1 <= k_n);
  q8 = 64*rm8 + d8p1.  One accumulated psum [96,22] (p = 12n+j):
     acc = 4096*U96 @ rm8 + SEL96 @ q8
         = 4096*stn(n_p,r) + 64*rm8(n_p,r) + d8p1(n_p,r)
  onehot[p,r] = (acc == 65+j_p) fires exactly for the last-writing
  (batch,j) pair of each covered output row (values are small ints,
  everything is exact).  out[22,7] = onehot^T @ x7 as one fp32 matmul
  (single addend per row -> exact), x7 columns DMAd straight from the
  full DRAM tensors, batch-id column via an SBUF->SBUF DMA from an iota
  row.  One direct 22-row DMA out; rows 22..99 stay at the runtime's
  zero-donated value.

All constants come from multi-level iota patterns; the [8,96] masks are
derived on DVE during the num_dets DMA window.  No constant inputs, no
scalar activation table load, no indirect DMA, no mod/divide ALU ops.
"""

import sys

import numpy as np

_TRN_REPO = "/opt/trn_rl_repo"
if _TRN_REPO not in sys.path:
    sys.path.insert(0, _TRN_REPO)

import concourse.bacc as bacc
import concourse.bass as bass
import concourse.mybir as mybir
import concourse.tile as tile
from concourse.bass_utils import run_bass_kernel_spmd

B = 8          # batches
N_FULL = 8192  # detections per batch in the full input
J = 12         # num_dets < 12, so only rows [:12] of each batch matter
R = 22         # off+k <= 11+11, so only out rows 0..21 are writable
R_FULL = 100   # fixed output rows
P96 = B * J    # 96 stacked (batch, j) source rows
GS = 4096.0    # suffix-count weight in the accumulated psum
GC = 64.0      # coverage weight (64 > max d8p1 = 22)

F32 = mybir.dt.float32
BF16 = mybir.dt.bfloat16
I32 = mybir.dt.int32


def _build_nc() -> bass.Bass:
    nc = bacc.Bacc(None, target_bir_lowering=False, num_swdge_queues=1)
    nd_d = nc.dram_tensor("num_dets", [B], I32, kind="ExternalInput")
    boxes_d = nc.dram_tensor("boxes", [B, N_FULL, 4], F32, kind="ExternalInput")
    scores_d = nc.dram_tensor("scores", [B, N_FULL], F32, kind="ExternalInput")
    classes_d = nc.dram_tensor("classes", [B, N_FULL], F32, kind="ExternalInput")
    out_d = nc.dram_tensor("out", [R_FULL, 7], F32, kind="ExternalOutput")

    alu = mybir.AluOpType

    with tile.TileContext(nc) as tc:
        with (
            tc.tile_pool(name="sb", bufs=1) as sb,
            tc.tile_pool(name="ps", bufs=1, space=bass.MemorySpace.PSUM) as ps,
        ):
            kb = sb.tile([B, 2], I32)
            kbf = sb.tile([B, 2], F32)
            r8i1 = sb.tile([B, R], I32)
            vdf96 = sb.tile([B, P96], F32)
            mch96 = sb.tile([B, P96], F32)
            jf96 = sb.tile([B, P96], F32)
            sel96 = sb.tile([B, P96], BF16)
            u96t = sb.tile([B, P96], F32)
            u96w = sb.tile([B, P96], BF16)
            jselp1 = sb.tile([B, P96], BF16)
            ones8 = sb.tile([B, 1], BF16)
            mar8 = sb.tile([B, 1], I32)
            mar8b = sb.tile([B, 1], BF16)
            jc96 = sb.tile([P96, 1], F32)
            x7 = sb.tile([P96, 7], F32)
            d8p1 = sb.tile([B, R], F32)
            t0 = sb.tile([B, R], F32)
            rm8 = sb.tile([B, R], BF16)
            q8 = sb.tile([B, R], BF16)
            onehot = sb.tile([P96, R], F32)
            outs = sb.tile([R, 7], F32)

            j96p1p = ps.tile([P96, 1], F32)
            vd96p = ps.tile([P96, 1], F32)
            acc96p = ps.tile([P96, R], F32)
            outp = ps.tile([R, 7], F32)

            # GpSimd: zero-fill + iota constants (num_dets DMA window)
            nc.gpsimd.memset(kb[:], 0)
            nc.gpsimd.memset(ones8[:], 1.0)
            nc.gpsimd.iota(
                vdf96[:], pattern=[[1, B], [0, J]], base=0, channel_multiplier=0,
                allow_small_or_imprecise_dtypes=True,
            )
            nc.gpsimd.iota(
                mch96[:], pattern=[[0, P96]], base=0, channel_multiplier=1,
                allow_small_or_imprecise_dtypes=True,
            )
            nc.gpsimd.iota(
                jf96[:], pattern=[[0, B], [1, J]], base=0, channel_multiplier=0,
                allow_small_or_imprecise_dtypes=True,
            )
            nc.gpsimd.iota(r8i1[:], pattern=[[1, R]], base=1, channel_multiplier=0)
            nc.gpsimd.iota(mar8[:], pattern=[[1, 1]], base=0, channel_multiplier=1)
            nc.gpsimd.tensor_copy(mar8b[:], mar8[:])

            # num_dets lands twice: col 0 = k_n, col 1 partitions 1..7 =
            # num_dets[n-1] (off_n); partition 0 of col 1 stays zeroed
            nc.sync.dma_start(
                out=kb[:, 0:1], in_=nd_d[:].rearrange("(p f) -> p f", f=1)
            )
            nc.scalar.dma_start(
                out=kb[1:B, 1:2], in_=nd_d[0 : B - 1].rearrange("(p f) -> p f", f=1)
            )
            nc.sync.dma_start(out=x7[:, 5:6], in_=classes_d[:, 0:J])
            nc.scalar.dma_start(out=x7[:, 1:5], in_=boxes_d[:, 0:J, :])
            nc.scalar.dma_start(out=x7[:, 6:7], in_=scores_d[:, 0:J])

            vec = nc.vector
            # DVE: mask constants (still inside the num_dets window)
            vec.tensor_tensor(sel96[:], vdf96[:], mch96[:], alu.is_equal)
            vec.tensor_tensor(u96t[:], mch96[:], vdf96[:], alu.is_gt)
            vec.tensor_scalar(u96w[:], u96t[:], GS, None, alu.mult)
            vec.scalar_tensor_tensor(
                jselp1[:], jf96[:], 1.0, sel96[:], alu.add, alu.mult
            )
            # PE: per-partition j+1 and batch-id columns (psum)
            nc.tensor.matmul(j96p1p[:], jselp1[:], ones8[:], start=True, stop=True)
            nc.tensor.matmul(vd96p[:], sel96[:], mar8b[:], start=True, stop=True)

            # DVE critical chain: one cast, then d, coverage, weights
            vec.tensor_copy(kbf[:], kb[:])
            vec.tensor_scalar(d8p1[:], r8i1[:], kbf[:, 1:2], None, alu.subtract)
            vec.tensor_scalar(t0[:], d8p1[:], 0.0, None, alu.is_gt)
            vec.scalar_tensor_tensor(
                rm8[:], d8p1[:], kbf[:, 0:1], t0[:], alu.is_le, alu.mult
            )
            vec.scalar_tensor_tensor(
                q8[:], rm8[:], GC, d8p1[:], alu.mult, alu.add
            )
            vec.tensor_copy(x7[:, 0:1], vd96p[:])
            # accumulated selector psum: 4096*stn + 64*rm + d8p1
            nc.tensor.matmul(acc96p[:], u96w[:], rm8[:], start=True, stop=False)
            nc.tensor.matmul(acc96p[:], sel96[:], q8[:], start=False, stop=True)
            vec.tensor_scalar(jc96[:], j96p1p[:], GC, None, alu.add)
            vec.tensor_scalar(onehot[:], acc96p[:], jc96[:], None, alu.is_equal)
            # gather payload: out[r,:] = x7[winner(r),:] (exact fp32 matmul)
            nc.tensor.matmul(outp[:], onehot[:], x7[:], start=True, stop=True)
            vec.tensor_copy(outs[:], outp[:])
            nc.sync.dma_start(out=out_d[0:R, :], in_=outs[:])

    nc.finalize()
    return nc


_CACHE: dict = {}


def _get_built():
    if "nc" not in _CACHE:
        _CACHE["nc"] = _build_nc()
    return _CACHE["nc"]


def run(inputs: dict, trace: bool = False, **spmd_kwargs):
    """Run on all 8 cores with replicated inputs; returns (out, BassKernelResults)."""
    nc = _get_built()
    in_map = {
        "num_dets": np.ascontiguousarray(inputs["num_dets"], dtype=np.int32),
        "boxes": np.ascontiguousarray(inputs["boxes"], dtype=np.float32),
        "scores": np.ascontiguousarray(inputs["scores"], dtype=np.float32),
        "classes": np.ascontiguousarray(inputs["classes"], dtype=np.float32),
    }
    res = run_bass_kernel_spmd(
        nc,
        [dict(in_map) for _ in range(8)],
        core_ids=list(range(8)),
        trace=trace,
        **spmd_kwargs,
    )
    return res.results[0]["out"], res


def kernel(num_dets, boxes, scores, classes):
    out, _ = run(
        {"num_dets": num_dets, "boxes": boxes, "scores": scores, "classes": classes}
    )
    return out
